# revision 45
# baseline (speedup 1.0000x reference)
"""Trainium2 Bass kernel for causal GQA attention (nn_CausalAttention).

Full-input contract: kernel(**inputs) takes the complete unsharded inputs and
returns the full [B, S, H] output. Internally shards across 8 NeuronCores as
(batch b in {0,1}) x (head-group g in {0..3}); each core computes 8 query heads
/ 2 KV heads for one batch and a row-parallel partial o_proj; the host sums the
4 partials per batch.

Device kernel math (per core, all bf16 matmuls, f32 PSUM accumulation):
  Q_T/K_T produced transposed ([dq, t]) via lhsT=weight^T over hidden^T
  RoPE applied via a block-rotation matmul (R) + cos/sin elementwise combine
  scores_T tile = kT(64,128-tk-tile)^T-contraction vs qT(64, 512-tq)  [tk, tq]
  P = exp(scale*s + bias) on ScalarE over double-buffered 2-bank PSUM groups
  causal: tile skipping + diag masks + visible-column narrowing on diag tiles
  PV via lhsT=[V|ones] (M=65): row 64 accumulates the softmax denominator
  normalize: reciprocal + K=1 outer-product matmul broadcast into the same bank
  o_partial = yT-tiles^T-contraction vs Wo_g^T  -> PSUM -> SBUF -> DRAM
  Emission is software-pipelined: next-chunk projections and previous-chunk
  o_proj units are interleaved between attention heads as PE filler.
"""

import math
import sys

import numpy as np

try:
    import concourse.bass as _probe  # noqa: F401
except ImportError:
    sys.path.insert(0, "/opt/trn_rl_repo")

import ml_dtypes

BF16 = ml_dtypes.bfloat16

# problem config (hardcoded per contract)
B, S, H = 2, 2048, 2048
NUM_HEADS, NUM_KV_HEADS, D = 32, 8, 64
NCORES = 8
GROUPS = 4                    # head-groups = cores per batch
QH = NUM_HEADS // GROUPS      # 8 q heads per core
KVH = NUM_KV_HEADS // GROUPS  # 2 kv heads per core
DQ = QH * D                   # 512
DKV = KVH * D                 # 128

P = 128
EXP_BIAS = -4.0

LAST_RESULTS = None
_NC_CACHE = {}


def make_rot_matrix(dtype=np.float32):
    """R such that (R^T @ q_T) = rotate_half(q) in [o, t] layout, per 64-row
    head block, with rotate_half's minus sign folded in."""
    R = np.zeros((P, P), dtype=np.float32)
    half = D // 2  # 32
    for hb in range(P // D):  # two head blocks per 128 partitions
        o = hb * D
        for r in range(half):
            R[o + half + r, o + r] = -1.0  # out_top = ... - q_bot * sin
            R[o + r, o + half + r] = 1.0   # out_bot = ... + q_top * sin
    return R.astype(dtype)


def make_diag_masks(tqc, dtype=np.float32):
    """masks[i, m, j] = 1 if key-row i of diag tile m is visible to query j.
    Diag tile m of a chunk covers keys at offset m*128 within the chunk; keep
    iff 128m + i <= j."""
    ndiag = tqc // P
    masks = np.zeros((P, ndiag, tqc), dtype=np.float32)
    i = np.arange(P)[:, None]
    j = np.arange(tqc)[None, :]
    for m in range(ndiag):
        masks[:, m, :] = (i + P * m <= j).astype(np.float32)
    return masks.astype(dtype)


def build_attention_nc(S=S, H=H, TQC=512, n_repeat=1):
    """Build the single-core SPMD Bass program. TQC = query-chunk width.
    n_repeat > 1 repeats the compute body (timing experiments only)."""
    import concourse.bass as bass  # noqa: F401
    import concourse.mybir as mybir
    import concourse.tile as tile
    from concourse import bacc
    from contextlib import ExitStack

    bf = mybir.dt.bfloat16
    f32 = mybir.dt.float32
    KT = H // P          # contraction tiles over hidden dim
    NTQ = S // TQC       # query chunks
    NTK = S // P         # key tiles
    GRP = TQC // P       # tiles per chunk == number of diag masks
    GRPE = 2             # tk tiles per exp group (2 PSUM banks, double-buffered)
    NQO = DQ // P        # q output 128-tiles (4)
    NHO = H // TQC       # o_proj output chunks (4)
    GQA = QH // KVH      # q heads per kv head (4)
    scale = 1.0 / math.sqrt(D)

    nc = bacc.Bacc()
    hidT = nc.declare_dram_parameter("hidT", [H, S], bf, isOutput=False)
    wqT = nc.declare_dram_parameter("wqT", [H, DQ], bf, isOutput=False)
    wkT = nc.declare_dram_parameter("wkT", [H, DKV], bf, isOutput=False)
    wvT = nc.declare_dram_parameter("wvT", [H, DKV], bf, isOutput=False)
    woT = nc.declare_dram_parameter("woT", [DQ, H], bf, isOutput=False)
    cosr = nc.declare_dram_parameter("cosr", [P, S], bf, isOutput=False)
    sinr = nc.declare_dram_parameter("sinr", [P, S], bf, isOutput=False)
    masks = nc.declare_dram_parameter("masks", [P, GRP, TQC], bf, isOutput=False)
    rmat = nc.declare_dram_parameter("rmat", [P, P], bf, isOutput=False)
    out = nc.declare_dram_parameter("out", [S, H], f32, isOutput=True)

    MUL = mybir.AluOpType.mult
    ADD = mybir.AluOpType.add
    EXP = mybir.ActivationFunctionType.Exp

    with ExitStack() as ctx:
        tc = ctx.enter_context(tile.TileContext(nc))
        const = ctx.enter_context(tc.tile_pool(name="const", bufs=1))
        hidp = ctx.enter_context(tc.tile_pool(name="hidp", bufs=2))
        work = ctx.enter_context(tc.tile_pool(name="work", bufs=4))
        ppool = ctx.enter_context(tc.tile_pool(name="psmall", bufs=2, space="PSUM"))
        pbig = ctx.enter_context(tc.tile_pool(name="pbig", bufs=2, space="PSUM"))
        po = ctx.enter_context(tc.tile_pool(name="po", bufs=2, space="PSUM"))

        wq_sb = const.tile([P, KT, DQ], bf)
        wk_sb = const.tile([P, KT, DKV], bf)
        wv_sb = const.tile([P, KT, DKV], bf)
        wo_sb = const.tile([P, NQO, H], bf)
        cos_sb = const.tile([P, S], bf)
        sin_sb = const.tile([P, S], bf)
        mask_sb = const.tile([P, GRP, TQC], bf)
        R_sb = const.tile([P, P], bf)
        # qT/kT live duplicated on partitions 0-63 and 64-127: the 64-row
        # QK matmuls are row-packed two-at-a-time onto PE tiles T0/T8
        # (tile_position (0,0)/(64,0)), and tile T8 reads both of its
        # operands from SBUF partitions 64-127.
        qT_sb = const.tile([P, QH, S], bf)
        kT_sb = const.tile([P, KVH, S], bf)
        vA_sb = [const.tile([P, NTK, 65], bf, name=f"vA{k}") for k in range(KVH)]
        yT_sb = const.tile([P, NQO, S], bf)

        # --- loads (SWDGE; ordered by first use — K/V weights feed the first
        # matmuls, wo is only needed ~40us in; hid chunks go via HWDGE in the
        # main loop so they stream in parallel with these) ---
        for ki in range(KT):
            nc.gpsimd.dma_start(wk_sb[:, ki, :], wkT[ki * P:(ki + 1) * P, :])
            nc.gpsimd.dma_start(wv_sb[:, ki, :], wvT[ki * P:(ki + 1) * P, :])
        nc.gpsimd.dma_start(R_sb[:], rmat[:])
        nc.gpsimd.dma_start(cos_sb[:], cosr[:])
        nc.gpsimd.dma_start(sin_sb[:], sinr[:])
        for ki in range(KT):
            nc.gpsimd.dma_start(wq_sb[:, ki, :], wqT[ki * P:(ki + 1) * P, :])
        nc.gpsimd.dma_start(mask_sb[:], masks[:])
        for f in range(NQO):
            nc.gpsimd.dma_start(wo_sb[:, f, :], woT[f * P:(f + 1) * P, :])
        for k in range(KVH):
            nc.vector.memset(vA_sb[k][:, :, 64:65], 1.0)
        exp_bias_sb = const.tile([P, 1], f32)
        nc.vector.memset(exp_bias_sb[:], EXP_BIAS)
        ones_col = const.tile([1, 64], bf)
        nc.vector.memset(ones_col[:], 1.0)

        def rope_project(ps, dest_writes, tq_sl):
            """ps: [P, TQC] psum with 2 heads of projected values (transposed).
            dest_writes: list of (dest_ap_for_half0, dest_ap_for_half1)."""
            raw = work.tile([P, TQC], bf, tag="rope_raw")
            nc.scalar.copy(raw, ps)
            rot_ps = ppool.tile([P, TQC], f32, tag="ps_proj")
            nc.tensor.matmul(rot_ps, R_sb[:], raw[:], start=True, stop=True)
            t_sin = work.tile([P, TQC], bf, tag="rope_sin")
            nc.vector.tensor_tensor(t_sin, rot_ps, sin_sb[:, tq_sl], MUL)
            t_cos = work.tile([P, TQC], bf, tag="rope_cos")
            nc.vector.tensor_tensor(t_cos, raw, cos_sb[:, tq_sl], MUL)
            for half, dest in enumerate(dest_writes):
                nc.vector.tensor_tensor(
                    dest,
                    t_cos[D * half:D * (half + 1), :],
                    t_sin[D * half:D * (half + 1), :],
                    ADD,
                )

        # --- per-chunk software pipeline with hand-interleaved filler work.
        # Within attention, ScalarE exp (573 ns/tile) is slower than PE's
        # QK+PV (426 ns/tile), so PE starves unless projection / o_proj
        # matmuls are interleaved between attention heads. Emission order:
        # chunk c's attention heads carry next-chunk projections and
        # previous-chunk o_proj units as PE filler.
        hid_chs = {}

        def emit_hid_load(c):
            tq = slice(c * TQC, (c + 1) * TQC)
            hid_ch = hidp.tile([P, KT, TQC], bf, tag="hid_ch")
            for ki in range(KT):
                nc.sync.dma_start(
                    hid_ch[:, ki, :], hidT[ki * P:(ki + 1) * P, tq]
                )
            hid_chs[c] = hid_ch

        def emit_k_proj(c):
            tq = slice(c * TQC, (c + 1) * TQC)
            ps_k = ppool.tile([P, TQC], f32, tag="ps_proj")
            for ki in range(KT):
                nc.tensor.matmul(
                    ps_k,
                    wk_sb[:, ki, :],
                    hid_chs[c][:, ki, :],
                    start=(ki == 0),
                    stop=(ki == KT - 1),
                )
            rope_project(ps_k, [kT_sb[0:D, 0, tq], kT_sb[0:D, 1, tq]], tq)
            nc.sync.dma_start(kT_sb[D:2 * D, :, tq], kT_sb[0:D, :, tq])

        def emit_v_proj(c, tt):
            tl = tt - c * GRP
            ps_v = ppool.tile([P, TQC], f32, tag="ps_proj")
            for ki in range(KT):
                nc.tensor.matmul(
                    ps_v[:, 0:DKV],
                    hid_chs[c][:, ki, tl * P:(tl + 1) * P],
                    wv_sb[:, ki, :],
                    start=(ki == 0),
                    stop=(ki == KT - 1),
                )
            for k in range(KVH):
                nc.vector.tensor_copy(
                    vA_sb[k][:, tt, 0:64], ps_v[:, k * D:(k + 1) * D]
                )

        def emit_q_proj(c, oo):
            tq = slice(c * TQC, (c + 1) * TQC)
            ps_q = ppool.tile([P, TQC], f32, tag="ps_proj")
            for ki in range(KT):
                nc.tensor.matmul(
                    ps_q,
                    wq_sb[:, ki, oo * P:(oo + 1) * P],
                    hid_chs[c][:, ki, :],
                    start=(ki == 0),
                    stop=(ki == KT - 1),
                )
            rope_project(
                ps_q,
                [qT_sb[0:D, 2 * oo, tq], qT_sb[0:D, 2 * oo + 1, tq]],
                tq,
            )
            nc.sync.dma_start(
                qT_sb[D:2 * D, 2 * oo:2 * oo + 2, tq],
                qT_sb[0:D, 2 * oo:2 * oo + 2, tq],
            )

        def emit_oproj_unit(c, tt, oc):
            ps_out = ppool.tile([P, TQC], f32, tag="ps_proj")
            for f in range(NQO):
                nc.tensor.matmul(
                    ps_out,
                    yT_sb[:, f, tt * P:(tt + 1) * P],
                    wo_sb[:, f, oc * TQC:(oc + 1) * TQC],
                    start=(f == 0),
                    stop=(f == NQO - 1),
                )
            o_sb = work.tile([P, TQC], f32, tag="o_stage")
            if (tt + oc) % 2 == 0:
                nc.vector.tensor_copy(o_sb[:], ps_out[:])
            else:
                nc.scalar.copy(o_sb[:], ps_out[:])
            nc.sync.dma_start(
                out[tt * P:(tt + 1) * P, oc * TQC:(oc + 1) * TQC], o_sb
            )

        def run_main_loop():
            for c in range(NTQ):
                _run_chunk(c)
            # final chunk's o_proj
            for tt in range((NTQ - 1) * GRP, NTQ * GRP):
                for oc in range(NHO):
                    emit_oproj_unit(NTQ - 1, tt, oc)

        def _run_chunk(c):
            tq_sl = slice(c * TQC, (c + 1) * TQC)
            if c + 1 < NTQ:
                emit_hid_load(c + 1)
            fillers = []
            if c + 1 < NTQ:
                fillers.append(lambda cc=c + 1: emit_k_proj(cc))
                for tt in range((c + 1) * GRP, (c + 2) * GRP):
                    fillers.append(lambda cc=c + 1, t=tt: emit_v_proj(cc, t))
                for oo in range(NQO):
                    fillers.append(lambda cc=c + 1, o=oo: emit_q_proj(cc, o))
            if c >= 1:
                for tt in range((c - 1) * GRP, c * GRP):
                    for oc in range(NHO):
                        fillers.append(
                            lambda cc=c - 1, t=tt, o=oc: emit_oproj_unit(cc, t, o)
                        )
            popped = 0

            # attention for this query chunk, all heads
            ntk = (c + 1) * GRP
            for h in range(QH):
                kv = h // GQA
                ps_o = po.tile([P, TQC], f32, tag="ps_o")
                for g in range(0, ntk, GRPE):
                    ps_s = pbig.tile([P, GRPE, TQC], f32, tag="ps_s")
                    # diag tiles (m >= 0) only have visible keys for queries
                    # j >= 128*m: restrict QK N-range and the exp span; the
                    # mask multiply zeroes the stale/masked region of p_sb.
                    los = []
                    for j in range(GRPE):
                        t = g + j
                        m = t - (ntk - GRP)
                        lo = max(0, m) * P if m >= 0 else 0
                        los.append(lo)
                        # row-pack the pair: j=0 on PE tile (0,0), j=1 on
                        # (64,0) via the partition-64 operand copies — the
                        # two 64-contraction matmuls stream concurrently
                        pb = D * (j % 2)
                        nc.tensor.matmul(
                            ps_s[:, j, lo:],
                            kT_sb[pb:pb + D, kv, t * P:(t + 1) * P],
                            qT_sb[pb:pb + D, h, c * TQC + lo:(c + 1) * TQC],
                            start=True,
                            stop=True,
                        )
                    p_sb = work.tile([P, GRPE, TQC], bf, tag="p_sb")
                    if all(lo == 0 for lo in los):
                        nc.scalar.activation(
                            p_sb[:], ps_s[:], EXP, bias=exp_bias_sb[:], scale=scale
                        )
                    else:
                        # diag group: per-tile exp over exactly the written span
                        for j in range(GRPE):
                            nc.scalar.activation(
                                p_sb[:, j, los[j]:],
                                ps_s[:, j, los[j]:],
                                EXP,
                                bias=exp_bias_sb[:],
                                scale=scale,
                            )
                    for j in range(GRPE):
                        t = g + j
                        m = t - (ntk - GRP)
                        if m >= 0:
                            lo = m * P
                            nc.vector.tensor_tensor(
                                p_sb[:, j, lo:],
                                p_sb[:, j, lo:],
                                mask_sb[:, m, lo:],
                                MUL,
                            )
                    for j in range(GRPE):
                        t = g + j
                        m = t - (ntk - GRP)
                        lo = max(0, m) * P if m >= 0 else 0
                        # diag tiles contribute nothing to queries j < 128m, so
                        # accumulate only the visible range (the first tile of
                        # each chunk is always full-width, initializing ps_o)
                        nc.tensor.matmul(
                            ps_o[0:65, lo:],
                            vA_sb[kv][:, t, :],
                            p_sb[:, j, lo:],
                            start=(t == 0),
                            stop=(t == ntk - 1),
                        )
                rec = work.tile([1, TQC], bf, tag="rec")
                with nc.allow_low_precision(
                    reason="1/l broadcast via bf16 matmul; ~0.2% uniform scale noise"
                ):
                    nc.vector.reciprocal(rec, ps_o[64:65, :])
                # broadcast 1/l to 64 partitions via a K=1 outer-product matmul
                # into the unused upper rows of the same PSUM bank (l row 64 is
                # dead once rec is computed)
                nc.tensor.matmul(
                    ps_o[64:128, :], ones_col[:], rec[:], start=True, stop=True
                )
                bc = work.tile([64, TQC], f32, tag="bc")
                nc.vector.tensor_copy(bc[:], ps_o[64:128, :])
                half = h % 2
                slot = h // 2
                nc.vector.tensor_tensor(
                    yT_sb[D * half:D * (half + 1), slot, tq_sl],
                    ps_o[0:64, :],
                    bc[:],
                    MUL,
                )
                # PE filler between heads: next-chunk projections and
                # previous-chunk o_proj units
                want = len(fillers) * (h + 1) // QH
                while popped < want:
                    fillers[popped]()
                    popped += 1

        for _rep in range(n_repeat):
            emit_hid_load(0)
            emit_k_proj(0)
            for tt in range(GRP):
                emit_v_proj(0, tt)
            for oo in range(NQO):
                emit_q_proj(0, oo)
            run_main_loop()

    nc.compile()
    return nc


def _get_nc():
    key = (S, H)
    if key not in _NC_CACHE:
        _NC_CACHE[key] = build_attention_nc()
    return _NC_CACHE[key]


def _prep_core_inputs(hidden_states, cos, sin, Wq, Wk, Wv, Wo):
    """Build the 8 per-core input maps (core index = b * GROUPS + g)."""
    TQC = 512
    masks_np = make_diag_masks(TQC, BF16)
    R_np = make_rot_matrix(BF16)
    in_maps = []
    per_batch = {}
    for b in range(B):
        hidT = np.ascontiguousarray(hidden_states[b].T).astype(BF16)
        cosr = np.ascontiguousarray(np.tile(cos[b].T, (2, 1))).astype(BF16)
        sinr = np.ascontiguousarray(np.tile(sin[b].T, (2, 1))).astype(BF16)
        per_batch[b] = (hidT, cosr, sinr)
    wq_g = [np.ascontiguousarray(Wq[DQ * g:DQ * (g + 1), :].T).astype(BF16)
            for g in range(GROUPS)]
    wk_g = [np.ascontiguousarray(Wk[DKV * g:DKV * (g + 1), :].T).astype(BF16)
            for g in range(GROUPS)]
    wv_g = [np.ascontiguousarray(Wv[DKV * g:DKV * (g + 1), :].T).astype(BF16)
            for g in range(GROUPS)]
    wo_g = [np.ascontiguousarray(Wo[:, DQ * g:DQ * (g + 1)].T).astype(BF16)
            for g in range(GROUPS)]
    for b in range(B):
        hidT, cosr, sinr = per_batch[b]
        for g in range(GROUPS):
            in_maps.append({
                "hidT": hidT,
                "wqT": wq_g[g],
                "wkT": wk_g[g],
                "wvT": wv_g[g],
                "woT": wo_g[g],
                "cosr": cosr,
                "sinr": sinr,
                "masks": masks_np,
                "rmat": R_np,
            })
    return in_maps


def kernel(hidden_states, cos, sin, Wq, Wk, Wv, Wo):
    global LAST_RESULTS
    from concourse.bass_utils import run_bass_kernel_spmd

    hidden_states = np.asarray(hidden_states, dtype=np.float32)
    cos = np.asarray(cos, dtype=np.float32)
    sin = np.asarray(sin, dtype=np.float32)
    Wq = np.asarray(Wq, dtype=np.float32)
    Wk = np.asarray(Wk, dtype=np.float32)
    Wv = np.asarray(Wv, dtype=np.float32)
    Wo = np.asarray(Wo, dtype=np.float32)
    assert hidden_states.shape == (B, S, H)

    nc = _get_nc()
    in_maps = _prep_core_inputs(hidden_states, cos, sin, Wq, Wk, Wv, Wo)
    res = run_bass_kernel_spmd(nc, in_maps, core_ids=list(range(NCORES)))
    LAST_RESULTS = res
    outs = [np.asarray(r["out"], dtype=np.float32) for r in res.results]
    full = np.empty((B, S, H), dtype=np.float32)
    for b in range(B):
        acc = outs[b * GROUPS]
        for g in range(1, GROUPS):
            acc = acc + outs[b * GROUPS + g]
        full[b] = acc
    return full
